# revision 62
# baseline (speedup 1.0000x reference)
"""Trainium2 Bass kernel for LogitBiasedSelfAttention1D.

Sharding: 8 cores = (batch b in 0..3) x (query half qh in 0..1).
Each core computes full attention (all 8 heads, all 2048 keys) for the
1024 queries of its batch half. No collectives.

Math decomposition (exactly equivalent to the reference up to fp):
  - conv1d key bias applied inside the softmax exp via the activation
    engine's per-partition bias operand: pt = exp(S + bias_key).
  - PV computed transposed (O^T = V^T P) so V is the stationary matmul
    operand: out psum rows 0..63 = head output (d, queries), row 64 =
    softmax denominator (V tile carries a ones column).
  - normalization: DVE reciprocal of the denominator row, stride-0
    broadcast DMA across partitions, DVE multiply into (c_in, queries)
    layout consumed directly by out_proj.
  - SCALE folded into w_q on host; b_out + residual folded into one
    host-prepared addend; LN gamma/beta folded into the final
    transpose drain.
All matmuls in bf16; accumulation and softmax denominator in fp32.
"""

import sys

for _p in ("/opt/trn_rl_repo", "/root/.axon_site/_ro/trn_rl_repo"):
    if _p not in sys.path:
        sys.path.insert(0, _p)

import numpy as np
import ml_dtypes

from concourse import bass, mybir
from concourse.tile import TileContext
from concourse.bass_utils import run_bass_kernel_spmd

B, C, T = 4, 512, 2048
H, D = 8, 64
SCALE = D ** -0.5
EPS = 1e-5
TQ = T // 2            # queries per core
KC = T // 128          # 16 key chunks
PAIRS = H // 2         # 4 head pairs
F32 = mybir.dt.float32
BF16 = mybir.dt.bfloat16
bf16 = ml_dtypes.bfloat16

Exp = mybir.ActivationFunctionType.Exp
Sqrt = mybir.ActivationFunctionType.Sqrt
Identity = mybir.ActivationFunctionType.Identity
Ln = mybir.ActivationFunctionType.Ln
MULT = mybir.AluOpType.mult
ADD = mybir.AluOpType.add

_CACHE = {}


def _build_nc():
    nc = bass.Bass()
    xct = nc.declare_dram_parameter("xct", [C, T], BF16, False)       # x[b], (C,T)
    xq = nc.declare_dram_parameter("xq", [C, TQ], BF16, False)        # query slice of x[b]
    xseq = nc.declare_dram_parameter("xseq", [TQ, C], F32, False)     # x[b].T slice + b_out
    wq = nc.declare_dram_parameter("wq", [C, C], BF16, False)         # (c_in, c_out), * SCALE
    wk = nc.declare_dram_parameter("wk", [C, C], BF16, False)
    wv = nc.declare_dram_parameter("wv", [C, C], BF16, False)
    wo = nc.declare_dram_parameter("wo", [C, C], BF16, False)
    cbp = nc.declare_dram_parameter("cb", [128, KC], F32, False)      # conv bias per key
    gmr = nc.declare_dram_parameter("gmr", [1, C], BF16, False)       # ln gamma row
    btr = nc.declare_dram_parameter("btr", [1, C], BF16, False)       # ln beta row
    outp = nc.declare_dram_parameter("out", [TQ, C], BF16, True)      # (tokens, C); host transposes+casts
    # DRAM scratch for the denominator-reciprocal row: a DRAM source AP may
    # broadcast with a stride-0 row (single coalesced descriptor), while an
    # SBUF source may not (64 descriptors, ~10us).
    scr = nc.declare_dram_parameter("scr", [1, 1024], F32, True)

    with TileContext(nc) as tc:
        with (
            tc.sbuf_pool(name="cst", bufs=1) as cst,
            tc.sbuf_pool(name="pex", bufs=1) as pex,
            tc.sbuf_pool(name="sml", bufs=1) as sml,
            tc.psum_pool(name="ps", bufs=1) as ps,
        ):
            # ---- constants / persistent tiles ----
            # One in-order DMA queue: critical-path inputs first, with the
            # XCT/WK pairs interleaved so the first KT matmul (needs
            # XCT[ci]+WK[ci]) can start before the whole x tensor lands.
            XCT, WK = [], []
            for i in range(4):
                XCT.append(cst.tile_from(xct[i * 128:(i + 1) * 128, :],
                                         name=f"XCT{i}"))
                WK.append(cst.tile_from(wk[i * 128:(i + 1) * 128, :],
                                        name=f"WK{i}"))
            WQ = [cst.tile_from(wq[i * 128:(i + 1) * 128, :], name=f"WQ{i}")
                  for i in range(4)]
            XQ = [cst.tile_from(xq[i * 128:(i + 1) * 128, :], name=f"XQ{i}")
                  for i in range(4)]
            WV = [cst.tile_from(wv[i * 128:(i + 1) * 128, :], name=f"WV{i}")
                  for i in range(4)]
            CB = cst.tile_from(cbp[:, :], name="CB")
            WO = [cst.tile_from(wo[i * 128:(i + 1) * 128, :], name=f"WO{i}")
                  for i in range(4)]
            XS = [cst.tile_from(xseq[t * 128:(t + 1) * 128, :], name=f"XS{t}")
                  for t in range(8)]
            # gamma/beta broadcast to all 128 partitions straight from DRAM
            # (channel runs along the free dim in the [tokens, C] tail layout;
            # bf16 so the gamma/beta tensor_tensor ops hit the DVE 2x mode)
            GB = cst.tile([128, C], BF16, name="GB")
            BB = cst.tile([128, C], BF16, name="BB")
            for row, tile in ((gmr, GB), (btr, BB)):
                rap = row[0:1, :]
                nc.sync.dma_start(out=tile[:, :], in_=bass.AP(
                    tensor=rap.tensor, offset=rap.offset, ap=[[0, 128], [1, C]]))

            epsT = cst.tile([128, 1], F32, name="epsT")
            nc.vector.memset(epsT[:, :], EPS)
            KT = [cst.tile([128, T], BF16, name=f"KT{m}") for m in range(4)]
            QT = [cst.tile([128, TQ], BF16, name=f"QT{m}") for m in range(4)]
            VB = [cst.tile([128, H * 65], BF16, name=f"VB{k}") for k in range(KC)]
            OT = [cst.tile([128, TQ], BF16, name=f"OTp{p}") for p in range(PAIRS)]

            # ones column per head (softmax denominator accumulator row)
            for k in range(KC):
                nc.vector.memset(
                    VB[k].rearrange("p (h e) -> p h e", e=65)[:, :, 64:65], 1.0)

            # ---- V projection first: its psum->sbuf copies ride the DVE
            # ahead of the attention loop (PV step k stalls on VB[k]) ----
            def emit_vb():
                for k in range(KC):
                    vps = ps.tile([128, 512], F32, tag="PO", bufs=2,
                                  name=f"vps{k}")
                    for ci in range(4):
                        nc.tensor.matmul(
                            vps[:, :],
                            lhsT=XCT[ci][:, k * 128:(k + 1) * 128],
                            rhs=WV[ci][:, :],
                            start=(ci == 0), stop=(ci == 3))
                    nc.vector.tensor_copy(
                        VB[k].rearrange("p (h e) -> p h e", e=65)[:, :, 0:64],
                        vps.rearrange("p (h e) -> p h e", e=64))

            # ---- K^T / Q^T projections for one pair-chunk m ----
            def emit_kt(m):
                for h2 in range(2):
                    kps = ps.tile([128, 1024], F32, tag="S", bufs=2,
                                  name=f"kps{m}_{h2}")
                    for n in range(2):
                        for ci in range(4):
                            nc.tensor.matmul(
                                kps[:, n * 512:(n + 1) * 512],
                                lhsT=WK[ci][:, m * 128:(m + 1) * 128],
                                rhs=XCT[ci][:, h2 * 1024 + n * 512: h2 * 1024 + (n + 1) * 512],
                                start=(ci == 0), stop=(ci == 3))
                    nc.vector.tensor_copy(KT[m][:, h2 * 1024:(h2 + 1) * 1024], kps[:, :])

            def emit_qt(m):
                qps = ps.tile([128, 1024], F32, tag="S", bufs=2, name=f"qps{m}")
                for n in range(2):
                    for ci in range(4):
                        nc.tensor.matmul(
                            qps[:, n * 512:(n + 1) * 512],
                            lhsT=WQ[ci][:, m * 128:(m + 1) * 128],
                            rhs=XQ[ci][:, n * 512:(n + 1) * 512],
                            start=(ci == 0), stop=(ci == 3))
                nc.vector.tensor_copy(QT[m][:, :], qps[:, :])

            def emit_norm(p, hi, po):
                # normalize: po rows 0..63 / row 64, into out_proj layout
                rden = sml.tile([1, 1024], F32, tag="rden", bufs=2,
                                name=f"rden{p}_{hi}")
                if p == PAIRS - 1 and hi == 1:
                    # last pair sits on the critical path into out_proj:
                    # reciprocal on the (now idle) ACT engine via
                    # 1/x = exp(-ln(x)); ln+exp share one act table so no
                    # table swap. The DVE reciprocal would cost ~6.5us here.
                    lden = sml.tile([1, 1024], F32, tag="lden", bufs=1,
                                    name=f"lden{p}_{hi}")
                    nc.scalar.activation(lden[:, :], po[64:65, :], Ln)
                    nc.scalar.activation(rden[:, :], lden[:, :], Exp,
                                         scale=-1.0)
                else:
                    nc.vector.reciprocal(rden[:, :], po[64:65, :])
                rb = sml.tile([64, 1024], F32, tag="rb", bufs=2,
                              name=f"rb{p}_{hi}")
                # partition-broadcast: bounce through DRAM so the
                # broadcast read is a stride-0-row DRAM descriptor.
                # Both DMAs sit on the same in-order queue, which
                # also serializes scratch-row reuse across pairs.
                nc.sync.dma_start(out=scr[0:1, :], in_=rden[0:1, :])
                scr_ap = scr[0:1, :]
                scr_bc = bass.AP(tensor=scr_ap.tensor,
                                 offset=scr_ap.offset,
                                 ap=[[0, 64], [1, 1024]])
                nc.sync.dma_start(out=rb[:, :], in_=scr_bc)
                nc.vector.tensor_tensor(
                    OT[p][hi * 64:(hi + 1) * 64, :],
                    po[0:64, :], rb[:, :], op=MULT)

            # ---- projections + early pair0/head0 scores ----
            # Head 0's scores+exp need only KT0/QT0, so they run while the
            # PE grinds through the V and remaining K/Q projections: the
            # ACT engine starts ~35us earlier than a fully serial head.
            # The 16 exp tiles are held in their own 16-deep pool; the PV
            # accumulation replays them once VB is ready. po for this head
            # must be allocated AFTER the vps tiles (same psum tag) or the
            # buffer rotation would deadlock against the V projection.
            # K/Q projections for pairs 1-3, on the PO psum tag so the
            # interleave never perturbs the attention s_ps rotation
            def emit_kt_po(m, h2):
                kps = ps.tile([128, 1024], F32, tag="PO", bufs=2,
                              name=f"kpo{m}_{h2}")
                for n in range(2):
                    for ci in range(4):
                        nc.tensor.matmul(
                            kps[:, n * 512:(n + 1) * 512],
                            lhsT=WK[ci][:, m * 128:(m + 1) * 128],
                            rhs=XCT[ci][:, h2 * 1024 + n * 512: h2 * 1024 + (n + 1) * 512],
                            start=(ci == 0), stop=(ci == 3))
                nc.vector.tensor_copy(KT[m][:, h2 * 1024:(h2 + 1) * 1024], kps[:, :])

            def emit_qt_po(m):
                qps = ps.tile([128, 1024], F32, tag="PO", bufs=2,
                              name=f"qpo{m}")
                for n in range(2):
                    for ci in range(4):
                        nc.tensor.matmul(
                            qps[:, n * 512:(n + 1) * 512],
                            lhsT=WQ[ci][:, m * 128:(m + 1) * 128],
                            rhs=XQ[ci][:, n * 512:(n + 1) * 512],
                            start=(ci == 0), stop=(ci == 3))
                nc.vector.tensor_copy(QT[m][:, :], qps[:, :])

            emit_kt(0)
            emit_qt(0)
            # batch BOTH pair-0 heads' scores+exp ahead of the PV replays.
            # Each ACT-bound batch carries PE filler so the tensor engine
            # stays warm: head 0 carries the V projection, head 1 carries
            # the pair 1-3 K/Q projections.
            ptb = {0: [], 1: []}
            for k in range(KC):
                s_ps = ps.tile([128, 1024], F32, tag="S", bufs=2,
                               name=f"s0_0_{k}")
                for n in range(2):
                    nc.tensor.matmul(
                        s_ps[:, n * 512:(n + 1) * 512],
                        lhsT=KT[0][0:64, k * 128:(k + 1) * 128],
                        rhs=QT[0][0:64, n * 512:(n + 1) * 512],
                        start=True, stop=True)
                pt = pex.tile([128, 1024], BF16, tag="pt0", bufs=KC,
                              name=f"pt0_{k}")
                nc.scalar.activation(pt[:, :], s_ps[:, :], Exp,
                                     bias=CB[:, k:k + 1])
                ptb[0].append(pt)
                vps = ps.tile([128, 512], F32, tag="PO", bufs=2,
                              name=f"vps{k}")
                for ci in range(4):
                    nc.tensor.matmul(
                        vps[:, :],
                        lhsT=XCT[ci][:, k * 128:(k + 1) * 128],
                        rhs=WV[ci][:, :],
                        start=(ci == 0), stop=(ci == 3))
                nc.vector.tensor_copy(
                    VB[k].rearrange("p (h e) -> p h e", e=65)[:, :, 0:64],
                    vps.rearrange("p (h e) -> p h e", e=64))
            groups = []
            for m in (1, 2, 3):
                groups += [lambda m=m: emit_kt_po(m, 0),
                           lambda m=m: emit_kt_po(m, 1),
                           lambda m=m: emit_qt_po(m)]
            gi = 0
            for k in range(KC):
                s_ps = ps.tile([128, 1024], F32, tag="S", bufs=2,
                               name=f"s0_1_{k}")
                for n in range(2):
                    nc.tensor.matmul(
                        s_ps[:, n * 512:(n + 1) * 512],
                        lhsT=KT[0][64:128, k * 128:(k + 1) * 128],
                        rhs=QT[0][64:128, n * 512:(n + 1) * 512],
                        start=True, stop=True)
                pt = pex.tile([128, 1024], BF16, tag="pt1", bufs=KC,
                              name=f"pt1_{k}")
                nc.scalar.activation(pt[:, :], s_ps[:, :], Exp,
                                     bias=CB[:, k:k + 1])
                ptb[1].append(pt)
                if gi < len(groups) and (k % 2 == 0 or gi >= 8):
                    groups[gi]()
                    gi += 1
            while gi < len(groups):
                groups[gi]()
                gi += 1
            for hi in range(2):
                po0 = ps.tile([128, 1024], F32, tag="PO", bufs=2,
                              name=f"po0_{hi}")
                for k in range(KC):
                    for n in range(2):
                        nc.tensor.matmul(
                            po0[0:65, n * 512:(n + 1) * 512],
                            lhsT=VB[k][:, hi * 65:(hi + 1) * 65],
                            rhs=ptb[hi][k][:, n * 512:(n + 1) * 512],
                            start=(k == 0), stop=(k == KC - 1))
                emit_norm(0, hi, po0)

            # ---- attention: per (pair, head-in-pair), O^T accumulation ----
            for p in range(PAIRS):
                for hi in range(2):
                    if p == 0:
                        continue
                    head = 2 * p + hi
                    rows = slice(hi * 64, (hi + 1) * 64)
                    po = ps.tile([128, 1024], F32, tag="PO", bufs=2,
                                 name=f"po{p}_{hi}")
                    for k in range(KC):
                        s_ps = ps.tile([128, 1024], F32, tag="S", bufs=2,
                                       name=f"s{p}_{hi}_{k}")
                        for n in range(2):
                            nc.tensor.matmul(
                                s_ps[:, n * 512:(n + 1) * 512],
                                lhsT=KT[p][rows, k * 128:(k + 1) * 128],
                                rhs=QT[p][rows, n * 512:(n + 1) * 512],
                                start=True, stop=True)
                        pt = pex.tile([128, 1024], BF16, tag="pt", bufs=2,
                                      name=f"pt{p}_{hi}_{k}")
                        nc.scalar.activation(pt[:, :], s_ps[:, :], Exp,
                                             bias=CB[:, k:k + 1])
                        for n in range(2):
                            nc.tensor.matmul(
                                po[0:65, n * 512:(n + 1) * 512],
                                lhsT=VB[k][:, head * 65:(head + 1) * 65],
                                rhs=pt[:, n * 512:(n + 1) * 512],
                                start=(k == 0), stop=(k == KC - 1))
                    emit_norm(p, hi, po)

            # ---- out_proj + residual + LN; gamma/beta on DVE in the
            # (tokens, C) layout, streamed straight to DRAM per chunk ----
            for t in range(8):
                ops_ = ps.tile([128, 512], F32, tag=("S", "PO")[t % 2], bufs=2,
                               name=f"op{t}")
                for p in range(PAIRS):
                    nc.tensor.matmul(
                        ops_[:, :],
                        lhsT=OT[p][:, t * 128:(t + 1) * 128],
                        rhs=WO[p][:, :],
                        start=(p == 0), stop=(p == PAIRS - 1))
                # residual (+b_out already folded in xseq)
                nc.vector.tensor_tensor(ops_[:, :], ops_[:, :], XS[t][:, :], op=ADD)
                bnst = sml.tile([128, 6], F32, tag="bnst", name=f"bnst{t}", bufs=3)
                bnag = sml.tile([128, 2], F32, tag="bnag", name=f"bnag{t}", bufs=3)
                nc.vector.bn_stats(bnst[:, :], ops_[:, :])
                nc.vector.bn_aggr(bnag[:, :], bnst[:, :])
                # rstd = (var+eps)^-0.5 = exp(-0.5*ln(var+eps)): stays in the
                # resident ln/exp act table (Sqrt would force a table swap)
                # and keeps the work off the tail-bottleneck DVE.
                lvar = sml.tile([128, 1], F32, tag="lvar", name=f"lvar{t}", bufs=3)
                nc.scalar.activation(lvar[:, :], bnag[:, 1:2], Ln, bias=epsT[:, :])
                rstd = sml.tile([128, 1], F32, tag="rstd", name=f"rstd{t}", bufs=3)
                nc.scalar.activation(rstd[:, :], lvar[:, :], Exp, scale=-0.5)
                nmr = sml.tile([128, 1], F32, tag="nmr", name=f"nmr{t}", bufs=3)
                nc.vector.tensor_scalar(out=nmr[:, :], in0=bnag[:, 0:1],
                                        scalar1=rstd[:, :], scalar2=-1.0,
                                        op0=MULT, op1=MULT)
                hn = sml.tile([128, C], BF16, tag="hn", name=f"hn{t}", bufs=3)
                nc.scalar.activation(hn[:, :], ops_[:, :], Identity,
                                     bias=nmr[:, :], scale=rstd[:, :])
                hg = sml.tile([128, C], BF16, tag="hg", name=f"hg{t}", bufs=3)
                nc.vector.tensor_tensor(hg[:, :], hn[:, :], GB[:, :], op=MULT)
                ho = sml.tile([128, C], BF16, tag="ho", name=f"ho{t}", bufs=3)
                nc.vector.tensor_tensor(ho[:, :], hg[:, :], BB[:, :], op=ADD)
                nc.sync.dma_start(out=outp[t * 128:(t + 1) * 128, :], in_=ho[:, :])

    _split_mm_waits(nc)
    return nc


def _split_mm_waits(nc):
    """Walrus MM structs carry only one sync wait; move extras to a NoOp."""
    f = nc.m.functions[0]
    for bb in f.blocks:
        il = bb.instructions
        out, changed = [], False
        for i in il:
            si = getattr(i, "sync_info", None)
            tn = type(i).__name__
            splittable = tn.startswith("Inst") and tn not in ("InstNoOp", "InstAllEngineBarrier")
            if (splittable and si is not None
                    and si.on_wait is not None and len(si.on_wait) > 1):
                waits = list(si.on_wait)
                for wi, w in enumerate(waits[:-1]):
                    out.append(mybir.InstNoOp(
                        name=f"{i.name}-wsplit{wi}", engine=i.engine,
                        sync_info=mybir.SyncInfo(on_wait=[w], on_update=[])))
                i.sync_info = mybir.SyncInfo(
                    on_wait=[waits[-1]], on_update=list(si.on_update))
                changed = True
            out.append(i)
        if changed:
            bb.instructions = out


def _prep_inputs(x, sqi, w_qkv, w_out, b_out, w_conv, b_conv, ln_gamma, ln_beta):
    x = np.asarray(x, np.float32)
    sqi = np.asarray(sqi, np.float32)
    w_qkv = np.asarray(w_qkv, np.float32)
    w_out = np.asarray(w_out, np.float32)
    b_out = np.asarray(b_out, np.float32)
    w_conv = np.asarray(w_conv, np.float32)
    b_conv = np.asarray(b_conv, np.float32)
    ln_gamma = np.asarray(ln_gamma, np.float32)
    ln_beta = np.asarray(ln_beta, np.float32)

    sp = np.pad(sqi, ((0, 0), (1, 1)))
    bias = (w_conv[0] * sp[:, :-2] + w_conv[1] * sp[:, 1:-1]
            + w_conv[2] * sp[:, 2:] + b_conv)                    # (B, T)

    wqT = (w_qkv[:C].T * SCALE).astype(bf16)
    wkT = w_qkv[C:2 * C].T.astype(bf16)
    wvT = w_qkv[2 * C:].T.astype(bf16)
    woT = w_out.T.astype(bf16)
    gmr = ln_gamma.reshape(1, C).astype(bf16)
    btr = ln_beta.reshape(1, C).astype(bf16)

    in_maps = []
    for core in range(8):
        b, qh = divmod(core, 2)
        qs = slice(qh * TQ, (qh + 1) * TQ)
        cb = bias[b].reshape(KC, 128).T.copy().astype(np.float32)
        in_maps.append({
            "xct": x[b].astype(bf16),
            "xq": x[b][:, qs].copy().astype(bf16),
            "xseq": (x[b].T[qs] + b_out).copy().astype(np.float32),
            "wq": wqT, "wk": wkT, "wv": wvT, "wo": woT,
            "cb": cb, "gmr": gmr, "btr": btr,
        })
    return in_maps


def kernel(x, sqi, w_qkv, w_out, b_out, w_conv, b_conv, ln_gamma, ln_beta,
           _trace=False):
    if "nc" not in _CACHE:
        _CACHE["nc"] = _build_nc()
    nc = _CACHE["nc"]
    in_maps = _prep_inputs(x, sqi, w_qkv, w_out, b_out, w_conv, b_conv,
                           ln_gamma, ln_beta)
    res = run_bass_kernel_spmd(nc, in_maps, core_ids=list(range(8)), trace=_trace)
    _CACHE["last_result"] = res
    out = np.empty((B, C, T), np.float32)
    for core in range(8):
        b, qh = divmod(core, 2)
        out[b][:, qh * TQ:(qh + 1) * TQ] = res.results[core]["out"].T.astype(np.float32)
    return out


# revision 63
# speedup vs baseline: 1.3098x; 1.3098x over previous
"""Trainium2 Bass kernel for LogitBiasedSelfAttention1D.

Sharding: 8 cores = (batch b in 0..3) x (query half qh in 0..1).
Each core computes full attention (all 8 heads, all 2048 keys) for the
1024 queries of its batch half. No collectives.

Math decomposition (exactly equivalent to the reference up to fp):
  - conv1d key bias applied inside the softmax exp via the activation
    engine's per-partition bias operand: pt = exp(S + bias_key).
  - PV computed transposed (O^T = V^T P) so V is the stationary matmul
    operand: out psum rows 0..63 = head output (d, queries), row 64 =
    softmax denominator (V tile carries a ones column).
  - normalization: DVE reciprocal of the denominator row, stride-0
    broadcast DMA across partitions, DVE multiply into (c_in, queries)
    layout consumed directly by out_proj.
  - SCALE folded into w_q on host; b_out + residual folded into one
    host-prepared addend; LN gamma/beta folded into the final
    transpose drain.
All matmuls in bf16; accumulation and softmax denominator in fp32.
"""

import sys

for _p in ("/opt/trn_rl_repo", "/root/.axon_site/_ro/trn_rl_repo"):
    if _p not in sys.path:
        sys.path.insert(0, _p)

import numpy as np
import ml_dtypes

from concourse import bass, mybir
from concourse.tile import TileContext
from concourse.bass_utils import run_bass_kernel_spmd

B, C, T = 4, 512, 2048
H, D = 8, 64
SCALE = D ** -0.5
EPS = 1e-5
TQ = T // 2            # queries per core
KC = T // 128          # 16 key chunks
PAIRS = H // 2         # 4 head pairs
F32 = mybir.dt.float32
BF16 = mybir.dt.bfloat16
bf16 = ml_dtypes.bfloat16

Exp = mybir.ActivationFunctionType.Exp
Sqrt = mybir.ActivationFunctionType.Sqrt
Identity = mybir.ActivationFunctionType.Identity
Ln = mybir.ActivationFunctionType.Ln
MULT = mybir.AluOpType.mult
ADD = mybir.AluOpType.add

_CACHE = {}


def _build_nc():
    nc = bass.Bass()
    xct = nc.declare_dram_parameter("xct", [C, T], BF16, False)       # x[b], (C,T)
    xq = nc.declare_dram_parameter("xq", [C, TQ], BF16, False)        # query slice of x[b]
    xseq = nc.declare_dram_parameter("xseq", [TQ, C], F32, False)     # x[b].T slice + b_out
    wq = nc.declare_dram_parameter("wq", [C, C], BF16, False)         # (c_in, c_out), * SCALE
    wk = nc.declare_dram_parameter("wk", [C, C], BF16, False)
    wv = nc.declare_dram_parameter("wv", [C, C], BF16, False)
    wo = nc.declare_dram_parameter("wo", [C, C], BF16, False)
    cbp = nc.declare_dram_parameter("cb", [128, KC], F32, False)      # conv bias per key
    gmr = nc.declare_dram_parameter("gmr", [1, C], BF16, False)       # ln gamma row
    btr = nc.declare_dram_parameter("btr", [1, C], BF16, False)       # ln beta row
    outp = nc.declare_dram_parameter("out", [TQ, C], BF16, True)      # (tokens, C); host transposes+casts
    # DRAM scratch for the denominator-reciprocal row: a DRAM source AP may
    # broadcast with a stride-0 row (single coalesced descriptor), while an
    # SBUF source may not (64 descriptors, ~10us).
    scr = nc.declare_dram_parameter("scr", [1, 1024], F32, True)

    with TileContext(nc) as tc:
        with (
            tc.sbuf_pool(name="cst", bufs=1) as cst,
            tc.sbuf_pool(name="pex", bufs=1) as pex,
            tc.sbuf_pool(name="sml", bufs=1) as sml,
            tc.psum_pool(name="ps", bufs=1) as ps,
        ):
            # ---- constants / persistent tiles ----
            # One in-order DMA queue: critical-path inputs first, with the
            # XCT/WK pairs interleaved so the first KT matmul (needs
            # XCT[ci]+WK[ci]) can start before the whole x tensor lands.
            XCT, WK = [], []
            for i in range(4):
                XCT.append(cst.tile_from(xct[i * 128:(i + 1) * 128, :],
                                         name=f"XCT{i}"))
                WK.append(cst.tile_from(wk[i * 128:(i + 1) * 128, :],
                                        name=f"WK{i}"))
            WQ = [cst.tile_from(wq[i * 128:(i + 1) * 128, :], name=f"WQ{i}")
                  for i in range(4)]
            XQ = [cst.tile_from(xq[i * 128:(i + 1) * 128, :], name=f"XQ{i}")
                  for i in range(4)]
            WV = [cst.tile_from(wv[i * 128:(i + 1) * 128, :], name=f"WV{i}")
                  for i in range(4)]
            CB = cst.tile_from(cbp[:, :], name="CB")
            WO = [cst.tile_from(wo[i * 128:(i + 1) * 128, :], name=f"WO{i}")
                  for i in range(4)]
            XS = [cst.tile_from(xseq[t * 128:(t + 1) * 128, :], name=f"XS{t}")
                  for t in range(8)]
            # gamma/beta broadcast to all 128 partitions straight from DRAM
            # (channel runs along the free dim in the [tokens, C] tail layout;
            # bf16 so the gamma/beta tensor_tensor ops hit the DVE 2x mode)
            GB = cst.tile([128, C], BF16, name="GB")
            BB = cst.tile([128, C], BF16, name="BB")
            for row, tile in ((gmr, GB), (btr, BB)):
                rap = row[0:1, :]
                nc.sync.dma_start(out=tile[:, :], in_=bass.AP(
                    tensor=rap.tensor, offset=rap.offset, ap=[[0, 128], [1, C]]))

            epsT = cst.tile([128, 1], F32, name="epsT")
            nc.vector.memset(epsT[:, :], EPS)
            KT = [cst.tile([128, T], BF16, name=f"KT{m}") for m in range(4)]
            QT = [cst.tile([128, TQ], BF16, name=f"QT{m}") for m in range(4)]
            VB = [cst.tile([128, H * 65], BF16, name=f"VB{k}") for k in range(KC)]
            OT = [cst.tile([128, TQ], BF16, name=f"OTp{p}") for p in range(PAIRS)]

            # ones column per head (softmax denominator accumulator row)
            for k in range(KC):
                nc.vector.memset(
                    VB[k].rearrange("p (h e) -> p h e", e=65)[:, :, 64:65], 1.0)

            # ---- V projection first: its psum->sbuf copies ride the DVE
            # ahead of the attention loop (PV step k stalls on VB[k]) ----
            def emit_vb():
                for k in range(KC):
                    vps = ps.tile([128, 512], F32, tag="PO", bufs=2,
                                  name=f"vps{k}")
                    for ci in range(4):
                        nc.tensor.matmul(
                            vps[:, :],
                            lhsT=XCT[ci][:, k * 128:(k + 1) * 128],
                            rhs=WV[ci][:, :],
                            start=(ci == 0), stop=(ci == 3))
                    nc.vector.tensor_copy(
                        VB[k].rearrange("p (h e) -> p h e", e=65)[:, :, 0:64],
                        vps.rearrange("p (h e) -> p h e", e=64))

            # ---- K^T / Q^T projections for one pair-chunk m ----
            def emit_kt(m):
                for h2 in range(2):
                    kps = ps.tile([128, 1024], F32, tag="S", bufs=2,
                                  name=f"kps{m}_{h2}")
                    for n in range(2):
                        for ci in range(4):
                            nc.tensor.matmul(
                                kps[:, n * 512:(n + 1) * 512],
                                lhsT=WK[ci][:, m * 128:(m + 1) * 128],
                                rhs=XCT[ci][:, h2 * 1024 + n * 512: h2 * 1024 + (n + 1) * 512],
                                start=(ci == 0), stop=(ci == 3))
                    nc.vector.tensor_copy(KT[m][:, h2 * 1024:(h2 + 1) * 1024], kps[:, :])

            def emit_qt(m):
                qps = ps.tile([128, 1024], F32, tag="S", bufs=2, name=f"qps{m}")
                for n in range(2):
                    for ci in range(4):
                        nc.tensor.matmul(
                            qps[:, n * 512:(n + 1) * 512],
                            lhsT=WQ[ci][:, m * 128:(m + 1) * 128],
                            rhs=XQ[ci][:, n * 512:(n + 1) * 512],
                            start=(ci == 0), stop=(ci == 3))
                nc.vector.tensor_copy(QT[m][:, :], qps[:, :])

            def emit_norm(p, hi, po):
                # normalize: po rows 0..63 / row 64, into out_proj layout
                rden = sml.tile([1, 1024], F32, tag="rden", bufs=2,
                                name=f"rden{p}_{hi}")
                if p == PAIRS - 1 and hi == 1:
                    # last pair sits on the critical path into out_proj:
                    # reciprocal on the (now idle) ACT engine via
                    # 1/x = exp(-ln(x)); ln+exp share one act table so no
                    # table swap. The DVE reciprocal would cost ~6.5us here.
                    lden = sml.tile([1, 1024], F32, tag="lden", bufs=1,
                                    name=f"lden{p}_{hi}")
                    nc.scalar.activation(lden[:, :], po[64:65, :], Ln)
                    nc.scalar.activation(rden[:, :], lden[:, :], Exp,
                                         scale=-1.0)
                else:
                    nc.vector.reciprocal(rden[:, :], po[64:65, :])
                rb = sml.tile([64, 1024], F32, tag="rb", bufs=2,
                              name=f"rb{p}_{hi}")
                # partition-broadcast: bounce through DRAM so the
                # broadcast read is a stride-0-row DRAM descriptor.
                # Both DMAs sit on the same in-order queue, which
                # also serializes scratch-row reuse across pairs.
                nc.sync.dma_start(out=scr[0:1, :], in_=rden[0:1, :])
                scr_ap = scr[0:1, :]
                scr_bc = bass.AP(tensor=scr_ap.tensor,
                                 offset=scr_ap.offset,
                                 ap=[[0, 64], [1, 1024]])
                nc.sync.dma_start(out=rb[:, :], in_=scr_bc)
                nc.vector.tensor_tensor(
                    OT[p][hi * 64:(hi + 1) * 64, :],
                    po[0:64, :], rb[:, :], op=MULT)

            # ---- projections + early pair0/head0 scores ----
            # Head 0's scores+exp need only KT0/QT0, so they run while the
            # PE grinds through the V and remaining K/Q projections: the
            # ACT engine starts ~35us earlier than a fully serial head.
            # The 16 exp tiles are held in their own 16-deep pool; the PV
            # accumulation replays them once VB is ready. po for this head
            # must be allocated AFTER the vps tiles (same psum tag) or the
            # buffer rotation would deadlock against the V projection.
            emit_kt(0)
            emit_qt(0)
            pt0s = []
            for k in range(KC):
                s_ps = ps.tile([128, 1024], F32, tag="S", bufs=2,
                               name=f"s0_0_{k}")
                for n in range(2):
                    nc.tensor.matmul(
                        s_ps[:, n * 512:(n + 1) * 512],
                        lhsT=KT[0][0:64, k * 128:(k + 1) * 128],
                        rhs=QT[0][0:64, n * 512:(n + 1) * 512],
                        start=True, stop=True)
                pt = pex.tile([128, 1024], BF16, tag="pt0", bufs=KC,
                              name=f"pt0_{k}")
                nc.scalar.activation(pt[:, :], s_ps[:, :], Exp,
                                     bias=CB[:, k:k + 1])
                pt0s.append(pt)
            emit_vb()
            for m in range(1, 4):
                emit_kt(m)
            for m in range(1, 4):
                emit_qt(m)
            po00 = ps.tile([128, 1024], F32, tag="PO", bufs=2, name="po0_0")
            for k in range(KC):
                for n in range(2):
                    nc.tensor.matmul(
                        po00[0:65, n * 512:(n + 1) * 512],
                        lhsT=VB[k][:, 0:65],
                        rhs=pt0s[k][:, n * 512:(n + 1) * 512],
                        start=(k == 0), stop=(k == KC - 1))
            emit_norm(0, 0, po00)

            # ---- attention: per (pair, head-in-pair), O^T accumulation ----
            for p in range(PAIRS):
                for hi in range(2):
                    if p == 0 and hi == 0:
                        continue
                    head = 2 * p + hi
                    rows = slice(hi * 64, (hi + 1) * 64)
                    po = ps.tile([128, 1024], F32, tag="PO", bufs=2,
                                 name=f"po{p}_{hi}")
                    for k in range(KC):
                        s_ps = ps.tile([128, 1024], F32, tag="S", bufs=2,
                                       name=f"s{p}_{hi}_{k}")
                        for n in range(2):
                            nc.tensor.matmul(
                                s_ps[:, n * 512:(n + 1) * 512],
                                lhsT=KT[p][rows, k * 128:(k + 1) * 128],
                                rhs=QT[p][rows, n * 512:(n + 1) * 512],
                                start=True, stop=True)
                        pt = pex.tile([128, 1024], BF16, tag="pt", bufs=2,
                                      name=f"pt{p}_{hi}_{k}")
                        nc.scalar.activation(pt[:, :], s_ps[:, :], Exp,
                                             bias=CB[:, k:k + 1])
                        for n in range(2):
                            nc.tensor.matmul(
                                po[0:65, n * 512:(n + 1) * 512],
                                lhsT=VB[k][:, head * 65:(head + 1) * 65],
                                rhs=pt[:, n * 512:(n + 1) * 512],
                                start=(k == 0), stop=(k == KC - 1))
                    emit_norm(p, hi, po)

            # ---- out_proj + residual + LN; gamma/beta on DVE in the
            # (tokens, C) layout, streamed straight to DRAM per chunk ----
            for t in range(8):
                ops_ = ps.tile([128, 512], F32, tag=("S", "PO")[t % 2], bufs=2,
                               name=f"op{t}")
                for p in range(PAIRS):
                    nc.tensor.matmul(
                        ops_[:, :],
                        lhsT=OT[p][:, t * 128:(t + 1) * 128],
                        rhs=WO[p][:, :],
                        start=(p == 0), stop=(p == PAIRS - 1))
                # residual (+b_out already folded in xseq)
                nc.vector.tensor_tensor(ops_[:, :], ops_[:, :], XS[t][:, :], op=ADD)
                bnst = sml.tile([128, 6], F32, tag="bnst", name=f"bnst{t}", bufs=3)
                bnag = sml.tile([128, 2], F32, tag="bnag", name=f"bnag{t}", bufs=3)
                nc.vector.bn_stats(bnst[:, :], ops_[:, :])
                nc.vector.bn_aggr(bnag[:, :], bnst[:, :])
                # rstd = (var+eps)^-0.5 = exp(-0.5*ln(var+eps)): stays in the
                # resident ln/exp act table (Sqrt would force a table swap)
                # and keeps the work off the tail-bottleneck DVE.
                lvar = sml.tile([128, 1], F32, tag="lvar", name=f"lvar{t}", bufs=3)
                nc.scalar.activation(lvar[:, :], bnag[:, 1:2], Ln, bias=epsT[:, :])
                rstd = sml.tile([128, 1], F32, tag="rstd", name=f"rstd{t}", bufs=3)
                nc.scalar.activation(rstd[:, :], lvar[:, :], Exp, scale=-0.5)
                nmr = sml.tile([128, 1], F32, tag="nmr", name=f"nmr{t}", bufs=3)
                nc.vector.tensor_scalar(out=nmr[:, :], in0=bnag[:, 0:1],
                                        scalar1=rstd[:, :], scalar2=-1.0,
                                        op0=MULT, op1=MULT)
                hn = sml.tile([128, C], BF16, tag="hn", name=f"hn{t}", bufs=3)
                nc.scalar.activation(hn[:, :], ops_[:, :], Identity,
                                     bias=nmr[:, :], scale=rstd[:, :])
                hg = sml.tile([128, C], BF16, tag="hg", name=f"hg{t}", bufs=3)
                nc.vector.tensor_tensor(hg[:, :], hn[:, :], GB[:, :], op=MULT)
                ho = sml.tile([128, C], BF16, tag="ho", name=f"ho{t}", bufs=3)
                nc.vector.tensor_tensor(ho[:, :], hg[:, :], BB[:, :], op=ADD)
                nc.sync.dma_start(out=outp[t * 128:(t + 1) * 128, :], in_=ho[:, :])

    _split_mm_waits(nc)
    return nc


def _split_mm_waits(nc):
    """Walrus MM structs carry only one sync wait; move extras to a NoOp."""
    f = nc.m.functions[0]
    for bb in f.blocks:
        il = bb.instructions
        out, changed = [], False
        for i in il:
            si = getattr(i, "sync_info", None)
            tn = type(i).__name__
            splittable = tn.startswith("Inst") and tn not in ("InstNoOp", "InstAllEngineBarrier")
            if (splittable and si is not None
                    and si.on_wait is not None and len(si.on_wait) > 1):
                waits = list(si.on_wait)
                for wi, w in enumerate(waits[:-1]):
                    out.append(mybir.InstNoOp(
                        name=f"{i.name}-wsplit{wi}", engine=i.engine,
                        sync_info=mybir.SyncInfo(on_wait=[w], on_update=[])))
                i.sync_info = mybir.SyncInfo(
                    on_wait=[waits[-1]], on_update=list(si.on_update))
                changed = True
            out.append(i)
        if changed:
            bb.instructions = out


def _prep_inputs(x, sqi, w_qkv, w_out, b_out, w_conv, b_conv, ln_gamma, ln_beta):
    x = np.asarray(x, np.float32)
    sqi = np.asarray(sqi, np.float32)
    w_qkv = np.asarray(w_qkv, np.float32)
    w_out = np.asarray(w_out, np.float32)
    b_out = np.asarray(b_out, np.float32)
    w_conv = np.asarray(w_conv, np.float32)
    b_conv = np.asarray(b_conv, np.float32)
    ln_gamma = np.asarray(ln_gamma, np.float32)
    ln_beta = np.asarray(ln_beta, np.float32)

    sp = np.pad(sqi, ((0, 0), (1, 1)))
    bias = (w_conv[0] * sp[:, :-2] + w_conv[1] * sp[:, 1:-1]
            + w_conv[2] * sp[:, 2:] + b_conv)                    # (B, T)

    wqT = (w_qkv[:C].T * SCALE).astype(bf16)
    wkT = w_qkv[C:2 * C].T.astype(bf16)
    wvT = w_qkv[2 * C:].T.astype(bf16)
    woT = w_out.T.astype(bf16)
    gmr = ln_gamma.reshape(1, C).astype(bf16)
    btr = ln_beta.reshape(1, C).astype(bf16)

    in_maps = []
    for core in range(8):
        b, qh = divmod(core, 2)
        qs = slice(qh * TQ, (qh + 1) * TQ)
        cb = bias[b].reshape(KC, 128).T.copy().astype(np.float32)
        in_maps.append({
            "xct": x[b].astype(bf16),
            "xq": x[b][:, qs].copy().astype(bf16),
            "xseq": (x[b].T[qs] + b_out).copy().astype(np.float32),
            "wq": wqT, "wk": wkT, "wv": wvT, "wo": woT,
            "cb": cb, "gmr": gmr, "btr": btr,
        })
    return in_maps


def kernel(x, sqi, w_qkv, w_out, b_out, w_conv, b_conv, ln_gamma, ln_beta,
           _trace=False):
    if "nc" not in _CACHE:
        _CACHE["nc"] = _build_nc()
    nc = _CACHE["nc"]
    in_maps = _prep_inputs(x, sqi, w_qkv, w_out, b_out, w_conv, b_conv,
                           ln_gamma, ln_beta)
    res = run_bass_kernel_spmd(nc, in_maps, core_ids=list(range(8)), trace=_trace)
    _CACHE["last_result"] = res
    out = np.empty((B, C, T), np.float32)
    for core in range(8):
        b, qh = divmod(core, 2)
        out[b][:, qh * TQ:(qh + 1) * TQ] = res.results[core]["out"].T.astype(np.float32)
    return out
